# revision 5
# baseline (speedup 1.0000x reference)
"""Trainium2 Bass kernel for nn_Node_GCN: out[n] = f(x[n]) + edge[n]^T @ g(cat(x,x)[n]).

Sharding: data-parallel over the batch dim N=8, one batch per NeuronCore.
Per core the dominant cost is streaming edge[n] from HBM once. edge and gx are
carried in fp8-e4m3 (fp32 PSUM accumulation), halving HBM traffic vs fp16 and
enabling DoubleRow matmuls (two 128-row contraction groups per instruction).
Edge rows are pair-interleaved on the host (partition p holds rows 2p, 2p+1 of
each 256-row block) so each pair block is one DMA with 4KB descriptors; the
matching gx layout comes from stride-2 stationary slices of h1g.

Queue split: the Sync queue dispatches the 8 edge-pair DMAs immediately while
the Scalar queue loads xT/weights/biases in parallel. Self-dynamics is kept
out of the edge accumulation group (separate PSUM, merged by DVE in the tail)
so the first edge matmul opens pout as soon as gx block 0 exists. Output is
stored fp16 (halves writeback; host casts up).

The device computes outT[n] = [h, j]; the host transposes to [j, h] while
unsharding.
"""

import numpy as np

D_IN = 64
D_HID = 128
M = 2048          # nodes per batch
N_BATCH = 8
NCORES = 8
NPAIR = M // 256  # 8 edge pair-blocks of 256 sender rows

# fp16 weights blob [128, WB_W]; rows 64:128 duplicate rows 0:64 for the
# K=64 matmuls whose rhs lives on partitions 64:128 (xT packed [128, 1024]).
_W_FW1 = 0          # f_w1 [64, 64]
_W_FW2 = 64         # f_w2 [64, 128]
_W_WG1 = 192        # wg1  [64, 128]  (= g_w1[:64] + g_w1[64:])
_W_GW2 = 320        # g_w2 [128, 128]
WB_W = 448

# fp32 bias blob [128, BB_W]
_B_GB2 = 0          # g_b2 broadcast rows, tiled 4x along free dim [128, 512]
_B_F1 = 512         # f_b1 [64, 1]
_B_G1 = 513         # g_b1 [128, 1]
_B_F2 = 514         # f_b2 [128, 1]
BB_W = 515

_NC_CACHE = {}


def _build():
    import concourse.bacc as bacc
    import concourse.mybir as mybir
    from concourse.tile import TileContext
    from concourse.bass import ts

    f32 = mybir.dt.float32
    f16 = mybir.dt.float16
    f8 = mybir.dt.float8e4
    AF = mybir.ActivationFunctionType
    DR = mybir.MatmulPerfMode.DoubleRow

    nc = bacc.Bacc()
    xT_d = nc.declare_dram_parameter("xT", [128, M // 2], f16, isOutput=False)
    # edge rows pair-interleaved by the host: row (256b + 2p + g) of edge[n]
    # lives at edge_d[b][p, g*2048: (g+1)*2048]
    edge_d = nc.declare_dram_parameter("edge", [M, M], f8, isOutput=False)
    wb_d = nc.declare_dram_parameter("wb", [128, WB_W], f16, isOutput=False)
    bb_d = nc.declare_dram_parameter("bb", [128, BB_W], f32, isOutput=False)
    outT_d = nc.declare_dram_parameter("outT", [D_HID, M], f16, isOutput=True)

    NCH = M // 512   # 4 chunks of 512 for wide matmuls / stores

    with TileContext(nc) as tc:
        with (
            tc.tile_pool(name="const", bufs=1) as cpool,
            tc.tile_pool(name="acts", bufs=1) as apool,
            tc.tile_pool(name="edgep", bufs=NPAIR) as epool,
            tc.tile_pool(name="pout", bufs=1, space="PSUM") as pout_pool,
            tc.tile_pool(name="pg", bufs=2, space="PSUM") as pg_pool,
            tc.tile_pool(name="pwork", bufs=2, space="PSUM") as pwork_pool,
        ):
            wb = cpool.tile([128, WB_W], f16, name="wb")
            bb = cpool.tile([128, BB_W], f32, name="bb")
            xT = cpool.tile([128, M // 2], f16, name="xT")

            # edge pair DMAs on the Sync queue, dispatched first so the big
            # stream starts as early as possible; 4KB descriptors (rows 2p,
            # 2p+1 contiguous per partition).
            ets = []
            for b in range(NPAIR):
                et = epool.tile([128, 2, M], f8, tag="e", name=f"et{b}")
                src = edge_d[256 * b:256 * b + 256, :].rearrange(
                    "(p g) j -> p g j", g=2
                )
                nc.sync.dma_start(out=et, in_=src)
                ets.append(et)

            # small loads on the Scalar queue, concurrent with the edge stream
            nc.scalar.dma_start(out=xT[:, 0:512], in_=xT_d[:, 0:512])
            nc.scalar.dma_start(out=wb, in_=wb_d[:])
            nc.scalar.dma_start(out=bb, in_=bb_d[:])
            nc.scalar.dma_start(out=xT[:, 512:1024], in_=xT_d[:, 512:1024])

            w_g2 = wb[0:128, _W_GW2:_W_GW2 + 128]
            w_f2 = wb[0:64, _W_FW2:_W_FW2 + 128]
            gb2b4 = bb[0:128, _B_GB2:_B_GB2 + 512]
            b_f1 = bb[0:64, _B_F1:_B_F1 + 1]
            b_g1 = bb[0:128, _B_G1:_B_G1 + 1]
            b_f2 = bb[0:128, _B_F2:_B_F2 + 1]

            h1f = apool.tile([D_IN, M], f16, name="h1f")
            h1g = apool.tile([D_HID, M], f16, name="h1g")
            # gx[p, 128*(2b+g) + h] = g(x)[sender 256b + 2p + g][h] in fp8
            gx = apool.tile([128, M], f8, name="gx")
            sdT = apool.tile([128, M], f16, name="sdT")
            outT = apool.tile([128, M], f16, name="outT")
            pout = pout_pool.tile([128, M], f32, name="pout")

            # token chunk k (tokens 512k..512k+511): xT partitions 64a..64a+64,
            # columns 512c2..512c2+512 with (a, c2) = divmod(k, 2)
            def mm_h1g(k):
                a, c2 = divmod(k, 2)
                w_g1 = wb[64 * a:64 * a + 64, _W_WG1:_W_WG1 + 128]
                rhs = xT[64 * a:64 * a + 64, ts(c2, 512)]
                psg = pg_pool.tile([128, 512], f32, tag="g", name="psg")
                nc.tensor.matmul(psg, w_g1, rhs, start=True, stop=True)
                return psg

            def act_h1g(psg, k):
                nc.scalar.activation(h1g[:, ts(k, 512)], psg, AF.Relu, bias=b_g1)

            def gx_chunk(k):
                # h1g chunk k -> gx tiles 4k..4k+3 (pair blocks 2k, 2k+1).
                # Stationary stride-2 token slices produce the pair-interleaved
                # sender order matching the edge DMA layout.
                psx = pwork_pool.tile([128, 512], f32, tag="w", name="psx")
                for t in range(4):
                    b, g = divmod(t, 2)
                    toks = h1g[:, 512 * k + 256 * b + g: 512 * k + 256 * (b + 1): 2]
                    nc.tensor.matmul(psx[:, ts(t, 128)], toks, w_g2,
                                     start=True, stop=True)
                nc.vector.tensor_add(gx[:, ts(k, 512)], psx, gb2b4)

            def h1f_chunk(k):
                a, c2 = divmod(k, 2)
                w_f1 = wb[64 * a:64 * a + 64, _W_FW1:_W_FW1 + 64]
                rhs = xT[64 * a:64 * a + 64, ts(c2, 512)]
                psf = pwork_pool.tile([64, 512], f32, tag="w", name="psf")
                nc.tensor.matmul(psf, w_f1, rhs, start=True, stop=True)
                nc.scalar.activation(h1f[:, ts(k, 512)], psf, AF.Relu, bias=b_f1)

            def sd_chunk(k):
                # self-dynamics kept out of pout: psd -> sdT fp16 (bias folded)
                psd = pg_pool.tile([128, 512], f32, tag="g", name="psd")
                nc.tensor.matmul(psd, w_f2, h1f[:, ts(k, 512)],
                                 start=True, stop=True)
                nc.scalar.activation(sdT[:, ts(k, 512)], psd, AF.Identity,
                                     bias=b_f2)

            def edge_block(b):
                et = ets[b]
                lhsT = gx[:, 256 * b:256 * b + 256].rearrange(
                    "p (g h) -> p g h", g=2
                )
                for c in range(NCH):
                    nc.tensor.matmul(
                        pout[:, ts(c, 512)], lhsT, et[:, :, ts(c, 512)],
                        start=(b == 0), stop=(b == NPAIR - 1),
                        perf_mode=DR,
                    )

            # PE order interleaves the MLP chain with edge blocks so edge
            # matmuls start as soon as gx block 0 exists and the PE stays busy
            # while later edge pairs stream in.
            g0 = mm_h1g(0)
            act_h1g(g0, 0)
            gx_chunk(0)
            edge_block(0)
            g1 = mm_h1g(1)
            act_h1g(g1, 1)
            gx_chunk(1)
            edge_block(1)
            g2 = mm_h1g(2)
            act_h1g(g2, 2)
            gx_chunk(2)
            edge_block(2)
            g3 = mm_h1g(3)
            act_h1g(g3, 3)
            gx_chunk(3)
            edge_block(3)
            # self-dynamics fills PE gaps while edge pairs 4..7 stream
            for k in range(NCH):
                h1f_chunk(k)
                sd_chunk(k)
            for b in range(4, NPAIR):
                edge_block(b)

            # tail: merge self-dynamics and store, chunk by chunk
            for c in range(NCH):
                nc.vector.tensor_add(outT[:, ts(c, 512)], pout[:, ts(c, 512)],
                                     sdT[:, ts(c, 512)])
                nc.sync.dma_start(out=outT_d[:, ts(c, 512)],
                                  in_=outT[:, ts(c, 512)])
    nc.compile()
    return nc


def _get_nc():
    if "nc" not in _NC_CACHE:
        _NC_CACHE["nc"] = _build()
    return _NC_CACHE["nc"]


def _prep_in_maps(inputs):
    import ml_dtypes

    f8 = ml_dtypes.float8_e4m3

    x = np.asarray(inputs["x"], dtype=np.float32)
    edge = np.asarray(inputs["edge"], dtype=np.float32)
    f_w1 = np.asarray(inputs["f_w1"], dtype=np.float32)
    f_b1 = np.asarray(inputs["f_b1"], dtype=np.float32)
    f_w2 = np.asarray(inputs["f_w2"], dtype=np.float32)
    f_b2 = np.asarray(inputs["f_b2"], dtype=np.float32)
    g_w1 = np.asarray(inputs["g_w1"], dtype=np.float32)
    g_b1 = np.asarray(inputs["g_b1"], dtype=np.float32)
    g_w2 = np.asarray(inputs["g_w2"], dtype=np.float32)
    g_b2 = np.asarray(inputs["g_b2"], dtype=np.float32)

    # cat(x, x) @ g_w1 == x @ (g_w1[:64] + g_w1[64:])
    wg1 = g_w1[:D_IN] + g_w1[D_IN:]

    wb = np.zeros((128, WB_W), dtype=np.float16)
    for r in (slice(0, 64), slice(64, 128)):  # duplicate for partition-64 rhs
        wb[r, _W_FW1:_W_FW1 + 64] = f_w1.astype(np.float16)
        wb[r, _W_FW2:_W_FW2 + 128] = f_w2.astype(np.float16)
        wb[r, _W_WG1:_W_WG1 + 128] = wg1.astype(np.float16)
    wb[0:128, _W_GW2:_W_GW2 + 128] = g_w2.astype(np.float16)

    bb = np.zeros((128, BB_W), dtype=np.float32)
    bb[0:128, _B_GB2:_B_GB2 + 512] = np.tile(g_b2[None, :], (128, 4))
    bb[0:64, _B_F1] = f_b1
    bb[0:128, _B_G1] = g_b1
    bb[0:128, _B_F2] = f_b2

    # x[n].T packed [128, 1024]: xT2[64a + k, t] = x[n, 1024a + t, k]
    xT = np.transpose(x, (0, 2, 1)).astype(np.float16)       # [8, 64, 2048]
    xT2 = np.concatenate([xT[:, :, :1024], xT[:, :, 1024:]], axis=1)
    xT2 = np.ascontiguousarray(xT2)

    # edge rows pair-interleaved within each 256-row block: DRAM row
    # 256b + 2p + g stays put (the interleave is expressed by the DMA access
    # pattern), so the host just casts to fp8.
    edge8 = edge.astype(f8)
    in_maps = [
        {
            "xT": xT2[n],
            "edge": np.ascontiguousarray(edge8[n]),
            "wb": wb,
            "bb": bb,
        }
        for n in range(N_BATCH)
    ]
    return in_maps


def run(inputs, trace=False, **kw):
    """Run on 8 cores; returns (out [8, 2048, 128] fp32, BassKernelResults)."""
    from concourse.bass_utils import run_bass_kernel_spmd

    nc = _get_nc()
    in_maps = _prep_in_maps(inputs)
    res = run_bass_kernel_spmd(nc, in_maps, list(range(NCORES)), trace=trace, **kw)
    outT = np.stack([np.asarray(res.results[n]["outT"]) for n in range(N_BATCH)])
    out = np.ascontiguousarray(np.transpose(outT, (0, 2, 1)))  # [8, 2048, 128]
    return out.astype(np.float32), res


def kernel(**inputs):
    out, _ = run(inputs, trace=False)
    return out


# revision 7
# speedup vs baseline: 1.1474x; 1.1474x over previous
"""Trainium2 Bass kernel for nn_Node_GCN: out[n] = f(x[n]) + edge[n]^T @ g(cat(x,x)[n]).

Sharding: data-parallel over the batch dim N=8, one batch per NeuronCore.
Per core the dominant cost is streaming edge[n] from HBM once. edge and gx are
carried in fp8-e4m3 (fp32 PSUM accumulation), halving HBM traffic vs fp16 and
running DoubleRow matmuls (two 128-row contraction groups per instruction, 1
output column/cycle -> 2x fp16 FLOP rate). Edge rows are pair-interleaved via
the DMA access pattern (partition p holds rows 2p, 2p+1 of each 256-row block)
so each pair block is one DMA with 4KB descriptors; the matching gx sender
order comes from stride-2 stationary slices of h1g.

All loads go on the Sync queue with the small MLP inputs FIRST (a parallel
queue gets starved by the edge stream), then the 8 edge-pair DMAs. Scratch
warm-up ops at t=0 ramp the PE clock and load the ACT function table before
real work arrives. Self-dynamics stays out of the edge accumulation group
(separate PSUM, merged by DVE in the tail) so the first edge matmul opens pout
as soon as gx block 0 exists. Output is stored fp16 (host casts up).

The device computes outT[n] = [h, j]; the host transposes to [j, h] while
unsharding.
"""

import numpy as np

D_IN = 64
D_HID = 128
M = 2048          # nodes per batch
N_BATCH = 8
NCORES = 8
NPAIR = M // 256  # 8 edge pair-blocks of 256 sender rows

# fp16 weights blob [128, WB_W]; rows 64:128 duplicate rows 0:64 for the
# K=64 matmuls whose rhs lives on partitions 64:128 (xT packed [128, 1024]).
_W_FW1 = 0          # f_w1 [64, 64]
_W_FW2 = 64         # f_w2 [64, 128]
_W_WG1 = 192        # wg1  [64, 128]  (= g_w1[:64] + g_w1[64:])
_W_GW2 = 320        # g_w2 [128, 128]
_W_GB2 = 448        # g_b2 broadcast rows, tiled 4x along free dim [128, 512]
WB_W = 960

# fp32 bias blob [128, 4]
_B_F1 = 0           # f_b1 [64, 1]
_B_G1 = 1           # g_b1 [128, 1]
_B_F2 = 2           # f_b2 [128, 1]
BB_W = 4

_NC_CACHE = {}


def _build():
    import concourse.bacc as bacc
    import concourse.mybir as mybir
    from concourse.tile import TileContext
    from concourse.bass import ts

    f32 = mybir.dt.float32
    f16 = mybir.dt.float16
    f8 = mybir.dt.float8e4
    AF = mybir.ActivationFunctionType
    DR = mybir.MatmulPerfMode.DoubleRow

    nc = bacc.Bacc()
    xT_d = nc.declare_dram_parameter("xT", [128, M // 2], f16, isOutput=False)
    edge_d = nc.declare_dram_parameter("edge", [M, M], f8, isOutput=False)
    wb_d = nc.declare_dram_parameter("wb", [128, WB_W], f16, isOutput=False)
    bb_d = nc.declare_dram_parameter("bb", [128, BB_W], f32, isOutput=False)
    outT_d = nc.declare_dram_parameter("outT", [D_HID, M], f16, isOutput=True)

    NCH = M // 512   # 4 chunks of 512 for wide matmuls

    with TileContext(nc) as tc:
        with (
            tc.tile_pool(name="const", bufs=1) as cpool,
            tc.tile_pool(name="acts", bufs=1) as apool,
            tc.tile_pool(name="edgep", bufs=NPAIR) as epool,
            tc.tile_pool(name="pout", bufs=1, space="PSUM") as pout_pool,
            tc.tile_pool(name="pg", bufs=2, space="PSUM") as pg_pool,
            tc.tile_pool(name="pwork", bufs=2, space="PSUM") as pwork_pool,
        ):
            wb = cpool.tile([128, WB_W], f16, name="wb")
            bb = cpool.tile([128, BB_W], f32, name="bb")
            xT = cpool.tile([128, M // 2], f16, name="xT")
            scratch = apool.tile([128, 512], f16, name="scratch")

            # MLP inputs first on the Sync queue (small loads lose DMA-engine
            # arbitration if they run beside the edge stream), then the edge
            # pair blocks back to back.
            nc.sync.dma_start(out=xT[:, 0:512], in_=xT_d[:, 0:512])
            nc.sync.dma_start(out=wb, in_=wb_d[:])
            nc.sync.dma_start(out=bb, in_=bb_d[:])
            nc.sync.dma_start(out=xT[:, 512:1024], in_=xT_d[:, 512:1024])
            ets = []
            for b in range(NPAIR):
                et = epool.tile([128, 2, M], f8, tag="e", name=f"et{b}")
                src = edge_d[256 * b:256 * b + 256, :].rearrange(
                    "(p g) j -> p g j", g=2
                )
                nc.sync.dma_start(out=et, in_=src)
                ets.append(et)

            w_g2 = wb[0:128, _W_GW2:_W_GW2 + 128]
            w_f2 = wb[0:64, _W_FW2:_W_FW2 + 128]
            gb2b4 = wb[0:128, _W_GB2:_W_GB2 + 512]
            b_f1 = bb[0:64, _B_F1:_B_F1 + 1]
            b_g1 = bb[0:128, _B_G1:_B_G1 + 1]
            b_f2 = bb[0:128, _B_F2:_B_F2 + 1]

            h1f = apool.tile([D_IN, M], f16, name="h1f")
            h1g = apool.tile([D_HID, M], f16, name="h1g")
            # gx[p, 128*(2b+g) + h] = g(x)[sender 256b + 2p + g][h] in fp8
            gx = apool.tile([128, M], f8, name="gx")
            sdT = apool.tile([128, M], f16, name="sdT")
            outT = apool.tile([128, M], f16, name="outT")
            pout = pout_pool.tile([128, M], f32, name="pout")

            # warm-ups: ~4 matmuls ramp the PE clock toward full speed; one
            # activation hoists the lazy ~1.3us ACT table load off the h1g
            # critical path. No DMA dependencies (scratch zeroed by DVE).
            warm2 = apool.tile([1, 1], f32, name="warm2")
            nc.vector.memset(scratch, 0)
            for _ in range(4):
                psw = pg_pool.tile([128, 512], f32, tag="g", name="psw")
                nc.tensor.matmul(psw, scratch[:, 0:128], scratch,
                                 start=True, stop=True)
            nc.scalar.activation(warm2, scratch[0:1, 0:1], AF.Relu, bias=0.0)

            # token chunk k (tokens 512k..512k+511): xT partitions 64a..64a+64,
            # columns 512c2..512c2+512 with (a, c2) = divmod(k, 2)
            def mm_h1g(k):
                a, c2 = divmod(k, 2)
                w_g1 = wb[64 * a:64 * a + 64, _W_WG1:_W_WG1 + 128]
                rhs = xT[64 * a:64 * a + 64, ts(c2, 512)]
                psg = pg_pool.tile([128, 512], f32, tag="g", name="psg")
                nc.tensor.matmul(psg, w_g1, rhs, start=True, stop=True)
                return psg

            def act_h1g(psg, k):
                nc.scalar.activation(h1g[:, ts(k, 512)], psg, AF.Relu, bias=b_g1)

            def gx_chunk(k):
                # h1g chunk k -> gx tiles 4k..4k+3 (pair blocks 2k, 2k+1).
                # Stationary stride-2 token slices produce the pair-interleaved
                # sender order matching the edge DMA layout.
                psx = pwork_pool.tile([128, 512], f32, tag="w", name="psx")
                for t in range(4):
                    b, g = divmod(t, 2)
                    toks = h1g[:, 512 * k + 256 * b + g: 512 * k + 256 * (b + 1): 2]
                    nc.tensor.matmul(psx[:, ts(t, 128)], toks, w_g2,
                                     start=True, stop=True)
                nc.vector.tensor_add(gx[:, ts(k, 512)], psx, gb2b4)

            def h1f_chunk(k):
                a, c2 = divmod(k, 2)
                w_f1 = wb[64 * a:64 * a + 64, _W_FW1:_W_FW1 + 64]
                rhs = xT[64 * a:64 * a + 64, ts(c2, 512)]
                psf = pwork_pool.tile([64, 512], f32, tag="w", name="psf")
                nc.tensor.matmul(psf, w_f1, rhs, start=True, stop=True)
                nc.scalar.activation(h1f[:, ts(k, 512)], psf, AF.Relu, bias=b_f1)

            def sd_chunk(k):
                # self-dynamics kept out of pout: psd -> sdT fp16 (bias folded)
                psd = pg_pool.tile([128, 512], f32, tag="g", name="psd")
                nc.tensor.matmul(psd, w_f2, h1f[:, ts(k, 512)],
                                 start=True, stop=True)
                nc.scalar.activation(sdT[:, ts(k, 512)], psd, AF.Identity,
                                     bias=b_f2)

            def edge_block(b):
                et = ets[b]
                lhsT = gx[:, 256 * b:256 * b + 256].rearrange(
                    "p (g h) -> p g h", g=2
                )
                for c in range(NCH):
                    nc.tensor.matmul(
                        pout[:, ts(c, 512)], lhsT, et[:, :, ts(c, 512)],
                        start=(b == 0), stop=(b == NPAIR - 1),
                        perf_mode=DR,
                    )

            # PE order interleaves the MLP chain with edge blocks so edge
            # matmuls start as soon as gx block 0 exists and the PE stays busy
            # while later edge pairs stream in.
            for k in range(NCH):
                psg = mm_h1g(k)
                act_h1g(psg, k)
                gx_chunk(k)
                edge_block(k)
            # self-dynamics fills PE gaps while edge pairs 4..7 stream
            for k in range(NCH):
                h1f_chunk(k)
                sd_chunk(k)
            for b in range(4, NPAIR):
                edge_block(b)

            # tail: merge self-dynamics and store in two 1024-wide chunks,
            # pipelined behind the last edge block's chunk matmuls
            for c in range(2):
                nc.vector.tensor_add(outT[:, ts(c, 1024)], pout[:, ts(c, 1024)],
                                     sdT[:, ts(c, 1024)])
                nc.sync.dma_start(out=outT_d[:, ts(c, 1024)],
                                  in_=outT[:, ts(c, 1024)])
    nc.compile()
    return nc


def _get_nc():
    if "nc" not in _NC_CACHE:
        _NC_CACHE["nc"] = _build()
    return _NC_CACHE["nc"]


def _prep_in_maps(inputs):
    import ml_dtypes

    f8 = ml_dtypes.float8_e4m3

    x = np.asarray(inputs["x"], dtype=np.float32)
    edge = np.asarray(inputs["edge"], dtype=np.float32)
    f_w1 = np.asarray(inputs["f_w1"], dtype=np.float32)
    f_b1 = np.asarray(inputs["f_b1"], dtype=np.float32)
    f_w2 = np.asarray(inputs["f_w2"], dtype=np.float32)
    f_b2 = np.asarray(inputs["f_b2"], dtype=np.float32)
    g_w1 = np.asarray(inputs["g_w1"], dtype=np.float32)
    g_b1 = np.asarray(inputs["g_b1"], dtype=np.float32)
    g_w2 = np.asarray(inputs["g_w2"], dtype=np.float32)
    g_b2 = np.asarray(inputs["g_b2"], dtype=np.float32)

    # cat(x, x) @ g_w1 == x @ (g_w1[:64] + g_w1[64:])
    wg1 = g_w1[:D_IN] + g_w1[D_IN:]

    wb = np.zeros((128, WB_W), dtype=np.float16)
    for r in (slice(0, 64), slice(64, 128)):  # duplicate for partition-64 rhs
        wb[r, _W_FW1:_W_FW1 + 64] = f_w1.astype(np.float16)
        wb[r, _W_FW2:_W_FW2 + 128] = f_w2.astype(np.float16)
        wb[r, _W_WG1:_W_WG1 + 128] = wg1.astype(np.float16)
    wb[0:128, _W_GW2:_W_GW2 + 128] = g_w2.astype(np.float16)
    wb[0:128, _W_GB2:_W_GB2 + 512] = np.tile(
        g_b2[None, :].astype(np.float16), (128, 4))

    bb = np.zeros((128, BB_W), dtype=np.float32)
    bb[0:64, _B_F1] = f_b1
    bb[0:128, _B_G1] = g_b1
    bb[0:128, _B_F2] = f_b2

    # x[n].T packed [128, 1024]: xT2[64a + k, t] = x[n, 1024a + t, k]
    xT = np.transpose(x, (0, 2, 1)).astype(np.float16)       # [8, 64, 2048]
    xT2 = np.concatenate([xT[:, :, :1024], xT[:, :, 1024:]], axis=1)
    xT2 = np.ascontiguousarray(xT2)

    # edge rows stay in natural order; the pair interleave is expressed by
    # the DMA access pattern, so the host just casts to fp8.
    edge8 = edge.astype(f8)
    in_maps = [
        {
            "xT": xT2[n],
            "edge": np.ascontiguousarray(edge8[n]),
            "wb": wb,
            "bb": bb,
        }
        for n in range(N_BATCH)
    ]
    return in_maps


def run(inputs, trace=False, **kw):
    """Run on 8 cores; returns (out [8, 2048, 128] fp32, BassKernelResults)."""
    from concourse.bass_utils import run_bass_kernel_spmd

    nc = _get_nc()
    in_maps = _prep_in_maps(inputs)
    res = run_bass_kernel_spmd(nc, in_maps, list(range(NCORES)), trace=trace, **kw)
    outT = np.stack([np.asarray(res.results[n]["outT"]) for n in range(N_BATCH)])
    out = np.ascontiguousarray(np.transpose(outT, (0, 2, 1)))  # [8, 2048, 128]
    return out.astype(np.float32), res


def kernel(**inputs):
    out, _ = run(inputs, trace=False)
    return out


# revision 15
# speedup vs baseline: 1.1567x; 1.0081x over previous
"""Trainium2 Bass kernel for nn_Node_GCN: out[n] = f(x[n]) + edge[n]^T @ g(cat(x,x)[n]).

Sharding: data-parallel over the batch dim N=8, one batch per NeuronCore.
Per core the dominant cost is streaming edge[n] from HBM once. edge and gx are
carried in fp8-e4m3 (fp32 PSUM accumulation), halving HBM traffic vs fp16 and
running DoubleRow matmuls (two 128-row contraction groups per instruction, 1
output column/cycle -> 2x fp16 FLOP rate). Edge rows are pair-interleaved via
the DMA access pattern (partition p holds rows 2p, 2p+1 of each 256-row block)
so each pair block is one DMA with 4KB descriptors; the matching gx sender
order comes from stride-2 stationary slices of h1g.

All loads go on the Sync queue with the small MLP inputs FIRST (a parallel
queue gets starved by the edge stream), then the 8 edge-pair DMAs. Scratch
warm-up ops at t=0 ramp the PE clock and load the ACT function table before
real work arrives. Self-dynamics stays out of the edge accumulation group
(separate PSUM, merged by DVE in the tail) so the first edge matmul opens pout
as soon as gx block 0 exists. Output is stored fp16 (host casts up).

The device computes outT[n] = [h, j]; the host transposes to [j, h] while
unsharding.
"""

import numpy as np

D_IN = 64
D_HID = 128
M = 2048          # nodes per batch
N_BATCH = 8
NCORES = 8
NPAIR = M // 256  # 8 edge pair-blocks of 256 sender rows

# single fp16 preamble blob [128, BL_W] with 4KB rows: xT + weights + g_b2
# broadcast. Rows 64:128 of the weight section duplicate rows 0:64 for the
# K=64 matmuls whose rhs lives on partitions 64:128 (xT packed [128, 1024]).
_BL_XT = 0          # xT [128, 1024]
_W_FW1 = 1024       # f_w1 [64, 64]
_W_FW2 = 1088       # f_w2 [64, 128]
_W_WG1 = 1216       # wg1  [64, 128]  (= g_w1[:64] + g_w1[64:])
_W_GW2 = 1344       # g_w2 [128, 128]
_W_GB2 = 1472       # g_b2 broadcast rows, tiled 4x along free dim [128, 512]
BL_W = 1984

# fp32 bias blob [128, 4]
_B_F1 = 0           # f_b1 [64, 1]
_B_G1 = 1           # g_b1 [128, 1]
_B_F2 = 2           # f_b2 [128, 1]
BB_W = 4

_NC_CACHE = {}


def _build():
    import concourse.bacc as bacc
    import concourse.mybir as mybir
    from concourse.tile import TileContext
    from concourse.bass import ts

    f32 = mybir.dt.float32
    f16 = mybir.dt.float16
    f8 = mybir.dt.float8e4
    AF = mybir.ActivationFunctionType
    DR = mybir.MatmulPerfMode.DoubleRow

    nc = bacc.Bacc()
    edge_d = nc.declare_dram_parameter("edge", [M, M], f8, isOutput=False)
    bl_d = nc.declare_dram_parameter("bl", [128, BL_W], f16, isOutput=False)
    bb_d = nc.declare_dram_parameter("bb", [128, BB_W], f32, isOutput=False)
    outT_d = nc.declare_dram_parameter("outT", [D_HID, M], f16, isOutput=True)

    NCH = M // 512   # 4 chunks of 512 for wide matmuls

    with TileContext(nc) as tc:
        with (
            tc.tile_pool(name="const", bufs=1) as cpool,
            tc.tile_pool(name="acts", bufs=1) as apool,
            tc.tile_pool(name="edgep", bufs=NPAIR) as epool,
            tc.tile_pool(name="pout", bufs=1, space="PSUM") as pout_pool,
            tc.tile_pool(name="pg", bufs=2, space="PSUM") as pg_pool,
            tc.tile_pool(name="pwork", bufs=2, space="PSUM") as pwork_pool,
        ):
            bl = cpool.tile([128, BL_W], f16, name="bl")
            bb = cpool.tile([128, BB_W], f32, name="bb")
            scratch = apool.tile([128, 512], f16, name="scratch")

            # One 4KB-row preamble blob first on the Sync queue (small loads
            # lose DMA-engine arbitration if they run beside the edge
            # stream), then the edge pair blocks back to back.
            nc.sync.dma_start(out=bl, in_=bl_d[:])
            nc.sync.dma_start(out=bb, in_=bb_d[:])
            ets = []
            for b in range(NPAIR):
                et = epool.tile([128, 2, M], f8, tag="e", name=f"et{b}")
                src = edge_d[256 * b:256 * b + 256, :].rearrange(
                    "(p g) j -> p g j", g=2
                )
                nc.sync.dma_start(out=et, in_=src)
                ets.append(et)

            xT = bl[0:128, _BL_XT:_BL_XT + 1024]
            w_g2 = bl[0:128, _W_GW2:_W_GW2 + 128]
            w_f2 = bl[0:64, _W_FW2:_W_FW2 + 128]
            gb2b4 = bl[0:128, _W_GB2:_W_GB2 + 512]
            b_f1 = bb[0:64, _B_F1:_B_F1 + 1]
            b_g1 = bb[0:128, _B_G1:_B_G1 + 1]
            b_f2 = bb[0:128, _B_F2:_B_F2 + 1]

            h1f = apool.tile([D_IN, M], f16, name="h1f")
            h1g = apool.tile([D_HID, M], f16, name="h1g")
            # gx[p, 128*(2b+g) + h] = g(x)[sender 256b + 2p + g][h] in fp8
            gx = apool.tile([128, M], f8, name="gx")
            sdT = apool.tile([128, M], f16, name="sdT")
            outT = apool.tile([128, M], f16, name="outT")
            pout = pout_pool.tile([128, M], f32, name="pout")

            # warm-ups: ~6 matmuls ramp the PE clock toward full speed; one
            # activation hoists the lazy ~1.3us ACT table load off the h1g
            # critical path. No DMA dependencies (scratch zeroed by DVE).
            warm2 = apool.tile([1, 1], f32, name="warm2")
            nc.vector.memset(scratch, 0)
            for _ in range(6):
                psw = pg_pool.tile([128, 512], f32, tag="g", name="psw")
                nc.tensor.matmul(psw, scratch[:, 0:128], scratch,
                                 start=True, stop=True)
            nc.scalar.activation(warm2, scratch[0:1, 0:1], AF.Relu, bias=0.0)

            # token chunk k (tokens 512k..512k+511): xT partitions 64a..64a+64,
            # columns 512c2..512c2+512 with (a, c2) = divmod(k, 2)
            def mm_h1g(k):
                a, c2 = divmod(k, 2)
                w_g1 = bl[64 * a:64 * a + 64, _W_WG1:_W_WG1 + 128]
                rhs = xT[64 * a:64 * a + 64, 512 * c2:512 * c2 + 512]
                psg = pg_pool.tile([128, 512], f32, tag="g", name="psg")
                nc.tensor.matmul(psg, w_g1, rhs, start=True, stop=True)
                nc.scalar.activation(h1g[:, ts(k, 512)], psg, AF.Relu, bias=b_g1)

            def gx_chunk(k):
                # h1g chunk k -> gx tiles 4k..4k+3 (pair blocks 2k, 2k+1).
                # Stationary stride-2 token slices produce the pair-interleaved
                # sender order matching the edge DMA layout.
                psx = pwork_pool.tile([128, 512], f32, tag="w", name="psx")
                for t in range(4):
                    b, g = divmod(t, 2)
                    toks = h1g[:, 512 * k + 256 * b + g: 512 * k + 256 * (b + 1): 2]
                    nc.tensor.matmul(psx[:, ts(t, 128)], toks, w_g2,
                                     start=True, stop=True)
                nc.vector.tensor_add(gx[:, ts(k, 512)], psx, gb2b4)

            def h1f_chunk(k):
                a, c2 = divmod(k, 2)
                w_f1 = bl[64 * a:64 * a + 64, _W_FW1:_W_FW1 + 64]
                rhs = xT[64 * a:64 * a + 64, 512 * c2:512 * c2 + 512]
                psf = pwork_pool.tile([64, 512], f32, tag="w", name="psf")
                nc.tensor.matmul(psf, w_f1, rhs, start=True, stop=True)
                nc.scalar.activation(h1f[:, ts(k, 512)], psf, AF.Relu, bias=b_f1)

            def sd_chunk(k):
                # self-dynamics kept out of pout: psd -> sdT fp16 (bias folded)
                psd = pg_pool.tile([128, 512], f32, tag="g", name="psd")
                nc.tensor.matmul(psd, w_f2, h1f[:, ts(k, 512)],
                                 start=True, stop=True)
                nc.scalar.activation(sdT[:, ts(k, 512)], psd, AF.Identity,
                                     bias=b_f2)

            def edge_block(b):
                et = ets[b]
                lhsT = gx[:, 256 * b:256 * b + 256].rearrange(
                    "p (g h) -> p g h", g=2
                )
                for c in range(NCH):
                    nc.tensor.matmul(
                        pout[:, ts(c, 512)], lhsT, et[:, :, ts(c, 512)],
                        start=(b == 0), stop=(b == NPAIR - 1),
                        perf_mode=DR,
                    )

            # PE emission order staggers each h1g chunk ahead of its
            # consumers (gx_k needs relu_k, which trails h1g_k on ACT by
            # ~0.7us) and slots edge blocks in as early as their gx tiles and
            # DMA data allow; h1f/sd fill the data-wait gaps mid-stream.
            mm_h1g(0)
            mm_h1g(1)
            gx_chunk(0)
            edge_block(0)
            mm_h1g(2)
            gx_chunk(1)
            edge_block(1)
            h1f_chunk(0)
            mm_h1g(3)
            gx_chunk(2)
            edge_block(2)
            h1f_chunk(1)
            gx_chunk(3)
            edge_block(3)
            h1f_chunk(2)
            h1f_chunk(3)
            edge_block(4)
            sd_chunk(0)
            edge_block(5)
            sd_chunk(1)
            edge_block(6)
            sd_chunk(2)
            sd_chunk(3)
            edge_block(7)

            # tail: merge self-dynamics chunk by chunk (DVE; Pool cannot read
            # PSUM), store in two 1024-wide chunks
            for c in range(NCH):
                nc.vector.tensor_add(outT[:, ts(c, 512)], pout[:, ts(c, 512)],
                                     sdT[:, ts(c, 512)])
                if c % 2 == 1:
                    nc.sync.dma_start(out=outT_d[:, ts(c // 2, 1024)],
                                      in_=outT[:, ts(c // 2, 1024)])
    nc.compile()
    return nc


def _get_nc():
    if "nc" not in _NC_CACHE:
        _NC_CACHE["nc"] = _build()
    return _NC_CACHE["nc"]


def _prep_in_maps(inputs):
    import ml_dtypes

    f8 = ml_dtypes.float8_e4m3

    x = np.asarray(inputs["x"], dtype=np.float32)
    edge = np.asarray(inputs["edge"], dtype=np.float32)
    f_w1 = np.asarray(inputs["f_w1"], dtype=np.float32)
    f_b1 = np.asarray(inputs["f_b1"], dtype=np.float32)
    f_w2 = np.asarray(inputs["f_w2"], dtype=np.float32)
    f_b2 = np.asarray(inputs["f_b2"], dtype=np.float32)
    g_w1 = np.asarray(inputs["g_w1"], dtype=np.float32)
    g_b1 = np.asarray(inputs["g_b1"], dtype=np.float32)
    g_w2 = np.asarray(inputs["g_w2"], dtype=np.float32)
    g_b2 = np.asarray(inputs["g_b2"], dtype=np.float32)

    # cat(x, x) @ g_w1 == x @ (g_w1[:64] + g_w1[64:])
    wg1 = g_w1[:D_IN] + g_w1[D_IN:]

    # x[n].T packed [128, 1024]: xT2[64a + k, t] = x[n, 1024a + t, k]
    xT = np.transpose(x, (0, 2, 1)).astype(np.float16)       # [8, 64, 2048]
    xT2 = np.concatenate([xT[:, :, :1024], xT[:, :, 1024:]], axis=1)

    bl = np.zeros((N_BATCH, 128, BL_W), dtype=np.float16)
    bl[:, :, _BL_XT:_BL_XT + 1024] = xT2
    for r in (slice(0, 64), slice(64, 128)):  # duplicate for partition-64 rhs
        bl[:, r, _W_FW1:_W_FW1 + 64] = f_w1.astype(np.float16)
        bl[:, r, _W_FW2:_W_FW2 + 128] = f_w2.astype(np.float16)
        bl[:, r, _W_WG1:_W_WG1 + 128] = wg1.astype(np.float16)
    bl[:, :, _W_GW2:_W_GW2 + 128] = g_w2.astype(np.float16)
    bl[:, :, _W_GB2:_W_GB2 + 512] = np.tile(
        g_b2[None, :].astype(np.float16), (128, 4))

    bb = np.zeros((128, BB_W), dtype=np.float32)
    bb[0:64, _B_F1] = f_b1
    bb[0:128, _B_G1] = g_b1
    bb[0:128, _B_F2] = f_b2

    # edge rows stay in natural order; the pair interleave is expressed by
    # the DMA access pattern, so the host just casts to fp8.
    edge8 = edge.astype(f8)
    in_maps = [
        {
            "bl": np.ascontiguousarray(bl[n]),
            "edge": np.ascontiguousarray(edge8[n]),
            "bb": bb,
        }
        for n in range(N_BATCH)
    ]
    return in_maps


def run(inputs, trace=False, **kw):
    """Run on 8 cores; returns (out [8, 2048, 128] fp32, BassKernelResults)."""
    from concourse.bass_utils import run_bass_kernel_spmd

    nc = _get_nc()
    in_maps = _prep_in_maps(inputs)
    res = run_bass_kernel_spmd(nc, in_maps, list(range(NCORES)), trace=trace, **kw)
    outT = np.stack([np.asarray(res.results[n]["outT"]) for n in range(N_BATCH)])
    out = np.ascontiguousarray(np.transpose(outT, (0, 2, 1)))  # [8, 2048, 128]
    return out.astype(np.float32), res


def kernel(**inputs):
    out, _ = run(inputs, trace=False)
    return out
